# revision 45
# baseline (speedup 1.0000x reference)
"""BWGNN (Bernstein-wavelet GNN) Trainium2 kernel, 8-core SPMD.

Sharding: nodes split 8 ways (graph/data parallel); edges partitioned by dst
shard; tiny weights replicated.  Per round of Laplacian message passing the
node-state table (dinv * f) is AllGathered, then per-edge src rows are
fetched with dma_gather (int16 indices, <=1024 per instruction - the HW
SWDGE ring cap).  Segment-sum by dst runs on the TensorEngine via one-hot
indicator matmuls (edges sorted by (src-quarter-chunk, dst-window), packed
contiguously; straddle tiles matmul once per window with self-masking
indicators built by the VectorEngine from iota-vs-dstw is_equal).

Pipeline overlap (this revision): each core's node shard is split into 4
quarters (3125 nodes + 75 zero-pad rows -> 3200 rows/quarter); the table
AllGather runs per (round, quarter) so chunk-c gathers only wait on their
own quarter's collective.  Edge chunks = src quarters.  Round r+1's table
build + AllGather for quarter q fire as soon as round r's last-chunk
bundles pass dst window 25(q+1) (bundles are window-sorted, so the quarter
is final), overlapping the collective and table math with the remaining
gathers.  The output filter/MLP chunks similarly start per quarter while
late round-2 bundles are still in flight.  Pad rows carry dinv=0 so table
pad rows are exactly zero (they double as the gather target for padded
edge slots).

MLP in/out runs feature-major with stationary-weight matmuls; node-major
states are produced by PE transposes; the three Bernstein filters are fused
into scaled-identity matmuls.  Outputs are written feature-major [64, sp];
the host transposes, de-pads and concatenates.

Gather padding is minimized by _balance_positions: per core, nodes are
LPT-packed into their quarter's windows so every window carries ~equal
in-degree per src-chunk; the canonical (chunk, window) group size is a
max over cores, so balancing cuts padded slots 6.3% -> 1.6% (200 bundles
of 1024 indices vs 208).  Within-quarter permutation keeps every node's
chunk membership, so this is a host-side change only.

Measured: 3.709 ms exec (neuron-profile; run-to-run DMA variance ~±0.1ms),
rel err 6.5e-7 vs the jax reference; prior session's baseline was 4.25 ms.
The wall is DMAGatherAnt descriptor generation on the Pool engine, measured
at ~470 ns + ~8.0 ns/index regardless of payload size or locality (2 rounds
x 200k edges/core -> ~3.4 ms busy at ~99% occupancy, zero mid-round gaps;
>1024 idx/instruction crashes the device).  Everything else hides under it:
head ~109 us (MLP quarter 0 + first collective), tail ~78 us.
"""

import sys
from contextlib import ExitStack

import numpy as np

try:
    import concourse  # noqa: F401
except ImportError:  # pragma: no cover
    sys.path.insert(0, "/opt/trn_rl_repo")

import concourse.bacc as bacc
import concourse.bass as bass
import concourse.mybir as mybir
import concourse.tile as tile
from concourse.bass_utils import run_bass_kernel_spmd
from concourse.library_config import mlp
from concourse.masks import make_identity

P = 128
F32 = mybir.dt.float32
I16 = mybir.dt.int16


class Cfg:
    def __init__(self, n_nodes, n_edges, in_feats, h_feats, n_cores,
                 max_span_tiles=8, mm_chunk=512):
        self.n_nodes, self.n_edges = n_nodes, n_edges
        self.in_feats, self.h = in_feats, h_feats
        self.nc = n_cores
        self.shard = n_nodes // n_cores          # 12500
        self.nq = 4                              # quarters per shard
        self.qdata = self.shard // self.nq       # 3125 real rows / quarter
        self.qrows = ((self.qdata + P - 1) // P) * P  # 3200 padded
        self.qt = self.qrows // P                # 25 tiles / quarter
        self.sp = self.qrows * self.nq           # 12800 padded shard
        self.t = self.sp // P                    # 100 node tiles
        self.n_chunks = self.nq                  # gather chunks = src quarters
        self.chunk = self.qrows * n_cores        # 25600 table rows / chunk
        assert self.chunk <= 32000, self.chunk   # int16 index headroom
        self.max_span_tiles = max_span_tiles
        self.mm_chunk = mm_chunk
        assert self.sp % mm_chunk == 0


def _pad_pos(cfg, local):
    """local node id within shard -> padded row position (unbalanced)."""
    return (local // cfg.qdata) * cfg.qrows + (local % cfg.qdata)


def _balance_positions(cfg, dst, src_chunk_of_edge):
    """Per core, permute nodes within their quarter so each of the quarter's
    windows carries ~equal in-degree PER SRC-CHUNK.  The canonical
    (chunk, window) gather-group size is max-over-cores, so balancing all
    four chunk-sums per window cuts the padded gather slots ~4x.  Within-
    quarter permutation keeps every node's own src-chunk membership (and so
    the whole chunk structure) unchanged.  Returns per-core position arrays
    [shard] -> padded position in [0, sp)."""
    shard_of = dst // cfg.shard
    pos_list = []
    wcap = np.full(cfg.qt, P, np.int64)
    wcap[-1] = cfg.qdata - (cfg.qt - 1) * P       # last window: 53 + pads
    for c in range(cfg.nc):
        m = shard_of == c
        dl = (dst[m] - c * cfg.shard).astype(np.int64)
        ch = src_chunk_of_edge[m]
        counts = np.bincount(dl * cfg.nq + ch,
                             minlength=cfg.shard * cfg.nq
                             ).reshape(cfg.shard, cfg.nq).astype(np.int64)
        pos = np.zeros(cfg.shard, np.int64)
        for q in range(cfg.nq):
            lo = q * cfg.qdata
            vecs = counts[lo:lo + cfg.qdata]
            order = np.argsort(-vecs.sum(1), kind="stable")
            sums = np.zeros((cfg.qt, cfg.nq), np.int64)
            fill = np.zeros(cfg.qt, np.int64)
            base = q * cfg.qrows
            for n in order:
                score = (sums + vecs[n]).max(axis=1)
                score[fill >= wcap] = 1 << 40
                w = int(np.argmin(score))
                pos[lo + n] = base + w * P + fill[w]
                fill[w] += 1
                sums[w] += vecs[n]
        pos_list.append(pos)
    return pos_list


# ---------------------------------------------------------------- host prep

def _per_core_groups(cfg, chunk_of, srcrow, dstpos):
    """-> dict (chunk, window) -> (src_row_i16[], dst_in_window_f32[])."""
    out = {}
    for c in range(cfg.n_chunks):
        m = chunk_of == c
        sc = srcrow[m].astype(np.int64)
        dc = dstpos[m].astype(np.int64)
        o = np.argsort(dc, kind="stable")
        sc, dc = sc[o], dc[o]
        w = dc // P
        for ww in np.unique(w):
            sel = w == ww
            out[(c, int(ww))] = (sc[sel].astype(np.int16),
                                 (dc[sel] % P).astype(np.float32))
    return out


def _wrap16(x):
    """flat int16 stream -> [128, n/16]: storage[p, col] = x[col*16 + p%16]."""
    assert len(x) % 16 == 0
    return np.tile(x.reshape(-1, 16).T, (8, 1)).copy()


def preprocess(cfg, in_feat, src, dst, W1, b1, W2, b2, W3, b3, W4, b4):
    n = cfg.n_nodes
    deg = np.bincount(dst, minlength=n).astype(np.float32)
    dinv = np.clip(deg, 1.0, None) ** -0.5

    src_local = src % cfg.shard
    chunk_of_e = src_local // cfg.qdata
    pos_list = _balance_positions(cfg, dst, chunk_of_e)
    pos_g = np.concatenate([pos_list[c] + c * cfg.sp for c in range(cfg.nc)])
    pos_src = pos_g[src] % cfg.sp
    srcrow_e = (src // cfg.shard) * cfg.qrows + (pos_src % cfg.qrows)

    shard_of = dst // cfg.shard
    groups = []
    for c in range(cfg.nc):
        m = shard_of == c
        dstpos = pos_g[dst[m]] - c * cfg.sp
        groups.append(_per_core_groups(cfg, chunk_of_e[m], srcrow_e[m],
                                       dstpos))

    # canonical (chunk, window) sizes = max over cores; groups packed
    # contiguously (tiles may straddle two windows; a straddle tile is
    # matmul'd once per window with self-masking indicators).
    keys = sorted(set().union(*[set(g.keys()) for g in groups]))
    sizes = {k: max(len(g.get(k, ((), ()))[0]) for g in groups) for k in keys}
    layout = {}
    chunk_len = {}
    for c in range(cfg.n_chunks):
        pos = 0
        tile_wins = {}
        for (cc, ww) in keys:
            if cc != c:
                continue
            t0 = pos // P
            if len(tile_wins.get(t0, ())) >= 2 and pos % P:
                pos = (t0 + 1) * P          # avoid 3-window tiles
            layout[(c, ww)] = (pos, pos + sizes[(c, ww)])
            for t in range(pos // P, (pos + sizes[(c, ww)] - 1) // P + 1):
                tile_wins.setdefault(t, []).append(ww)
            pos += sizes[(c, ww)]
        chunk_len[c] = -(-pos // P) * P
        layout[("tw", c)] = tile_wins

    # bundles of <= max_span_tiles tiles per chunk; per-bundle matmul events
    plan = []  # (chunk, btiles, goff_tiles, [(w, [(tile_in_bundle, sel)])])
    goff = 0
    for c in range(cfg.n_chunks):
        tiles = chunk_len[c] // P
        tile_wins = layout[("tw", c)]
        b0 = 0
        while b0 < tiles:
            bt = min(cfg.max_span_tiles, tiles - b0)
            events = {}
            for t in range(b0, b0 + bt):
                for si, ww in enumerate(tile_wins.get(t, [])):
                    events.setdefault(ww, []).append((t - b0, si + 1))
            plan.append((c, bt, goff, sorted(events.items())))
            goff += bt
            b0 += bt
    total_tiles = goff

    in_maps = []
    zero_row = cfg.qdata          # first pad row of core 0, in every chunk

    for core in range(cfg.nc):
        gz = np.zeros(total_tiles * P, np.int16)
        dw1 = np.full(total_tiles * P, 999.0, np.float32)
        dw2 = np.full(total_tiles * P, 999.0, np.float32)
        cbase = {}
        acc_t = 0
        for c in range(cfg.n_chunks):
            cbase[c] = acc_t * P
            acc_t += chunk_len[c] // P
        for (c, ww) in keys:
            start, end = layout[(c, ww)]
            s_arr, d_arr = groups[core].get(
                (c, ww), (np.zeros(0, np.int16), np.zeros(0, np.float32)))
            o = cbase[c] + start
            gz[o:o + (end - start)] = zero_row
            gz[o:o + len(s_arr)] = s_arr
            tile_wins = layout[("tw", c)]
            sl = np.arange(start, end)
            tl = sl // P
            first = np.array([tile_wins[t][0] for t in tl])
            dl_full = np.full(end - start, 999.0 + 128000.0, np.float64)
            dl_full[:len(d_arr)] = d_arr + 128.0 * ww
            v1 = dl_full - 128.0 * first
            v1[(v1 < 0) | (v1 >= P) | (dl_full > 90000)] = 999.0
            second = np.array([tile_wins[t][1] if len(tile_wins[t]) > 1
                               else -999 for t in tl])
            v2 = dl_full - 128.0 * second
            v2[(v2 < 0) | (v2 >= P) | (dl_full > 90000)] = 999.0
            dw1[o:o + (end - start)] = v1
            dw2[o:o + (end - start)] = v2
        lo = core * cfg.shard
        pos = pos_list[core]
        xT = np.zeros((cfg.in_feats, cfg.sp), np.float32)
        xT[:, pos] = in_feat[lo:lo + cfg.shard].T
        full = np.zeros(cfg.sp, np.float32)       # pads carry dinv=0
        full[pos] = dinv[lo:lo + cfg.shard]
        dpm = np.ascontiguousarray(full.reshape(cfg.t, P).T)
        dwt1 = np.ascontiguousarray(dw1.reshape(total_tiles, P).T)
        dwt2 = np.ascontiguousarray(dw2.reshape(total_tiles, P).T)
        in_maps.append({
            "xT": xT, "dinv_pm": dpm,
            "gidx": _wrap16(gz), "dstw1": dwt1, "dstw2": dwt2,
            "W1": np.asarray(W1, np.float32), "W2": np.asarray(W2, np.float32),
            "W3": np.asarray(W3, np.float32), "W4": np.asarray(W4, np.float32),
            "b1": np.asarray(b1, np.float32).reshape(-1, 1),
            "b2": np.asarray(b2, np.float32).reshape(-1, 1),
            "b3": np.asarray(b3, np.float32).reshape(-1, 1),
            "b4": np.asarray(b4, np.float32).reshape(-1, 1),
        })
    return in_maps, plan, total_tiles, pos_list


# ---------------------------------------------------------------- builder

def build_nc(cfg, plan, total_tiles):
    H = cfg.h
    NQ = cfg.nq
    QT = cfg.qt
    idx_cols = total_tiles * 8
    nc = bacc.Bacc("TRN2", target_bir_lowering=False, debug=False,
                   num_devices=cfg.nc)
    xT_d = nc.dram_tensor("xT", [cfg.in_feats, cfg.sp], F32, kind="ExternalInput")
    dinv_d = nc.dram_tensor("dinv_pm", [P, cfg.t], F32, kind="ExternalInput")
    gidx_d = nc.dram_tensor("gidx", [P, idx_cols], I16, kind="ExternalInput")
    dstw1_d = nc.dram_tensor("dstw1", [P, total_tiles], F32, kind="ExternalInput")
    dstw2_d = nc.dram_tensor("dstw2", [P, total_tiles], F32, kind="ExternalInput")
    W_d = {w: nc.dram_tensor(w, [cfg.in_feats if w in ("W1", "W4") else H, H],
                             F32, kind="ExternalInput")
           for w in ("W1", "W2", "W3", "W4")}
    b_d = {b: nc.dram_tensor(b, [H, 1], F32, kind="ExternalInput")
           for b in ("b1", "b2", "b3", "b4")}
    outl_d = nc.dram_tensor("out_l", [H, cfg.sp], F32, kind="ExternalOutput")
    outh_d = nc.dram_tensor("out_h", [H, cfg.sp], F32, kind="ExternalOutput")

    relu = mybir.ActivationFunctionType.Relu
    cp = mybir.ActivationFunctionType.Copy

    with tile.TileContext(nc) as tc, ExitStack() as ctx:
        pers = ctx.enter_context(tc.tile_pool(name="pers", bufs=1))
        dram = ctx.enter_context(tc.tile_pool(name="dram", bufs=1, space="DRAM"))
        io = ctx.enter_context(tc.tile_pool(name="io", bufs=2))
        xcp = ctx.enter_context(tc.tile_pool(name="xcp", bufs=4))
        idxp = ctx.enter_context(tc.tile_pool(name="idxp", bufs=6))
        gbp = ctx.enter_context(tc.tile_pool(name="gbp", bufs=6))
        gbi = ctx.enter_context(tc.tile_pool(name="gbi", bufs=2))
        psum = ctx.enter_context(tc.tile_pool(name="psum", bufs=2, space="PSUM"))
        psum1 = ctx.enter_context(tc.tile_pool(name="psum1", bufs=2, space="PSUM"))
        psum2 = ctx.enter_context(tc.tile_pool(name="psum2", bufs=2, space="PSUM"))

        nc.gpsimd.load_library(mlp)

        tblp = ctx.enter_context(tc.tile_pool(name="tblp", bufs=2))
        aggp = ctx.enter_context(tc.tile_pool(name="aggp", bufs=2))

        f0 = pers.tile([P, cfg.t, 64], F32, tag="f0")
        f1 = pers.tile([P, cfg.t, 64], F32, tag="f1")
        f2 = pers.tile([P, cfg.t, 64], F32, tag="f2")
        dinv_s = pers.tile([P, cfg.t], F32, tag="dinv")
        Ws = {w: pers.tile([cfg.in_feats if w in ("W1", "W4") else H, H],
                           F32, tag=w, name=w + "_s")
              for w in ("W1", "W2", "W3", "W4")}
        bs = {b: pers.tile([H, 1], F32, tag=b, name=b + "_s")
              for b in ("b1", "b2", "b3", "b4")}
        ident = pers.tile([P, P], F32, tag="ident")
        sid3 = pers.tile([P, P], F32, tag="sid3")
        sid075 = pers.tile([P, P], F32, tag="sid075")
        sidm15 = pers.tile([P, P], F32, tag="sidm15")

        tb_ins = [dram.tile([cfg.sp, 64], F32, name=f"tb_in{r}")
                  for r in range(2)]
        tb_fulls = [[dram.tile([cfg.chunk, 64], F32, addr_space="Shared",
                               name=f"tb_full{r}_{q}") for q in range(NQ)]
                    for r in range(2)]
        iota_f = pers.tile([P, P], F32, tag="iota_f")

        for w in Ws:
            nc.sync.dma_start(Ws[w][:], W_d[w][:])
        for b in bs:
            nc.sync.dma_start(bs[b][:], b_d[b][:])
        nc.sync.dma_start(dinv_s[:], dinv_d[:])
        make_identity(nc, ident[:])
        nc.vector.tensor_scalar_mul(sid3[:], ident[:], 3.0)
        nc.vector.tensor_scalar_mul(sid075[:], ident[:], 0.75)
        nc.vector.tensor_scalar_mul(sidm15[:], ident[:], -1.5)
        ioti = pers.tile([P, P], mybir.dt.int32, tag="ioti")
        nc.gpsimd.iota(ioti[:], pattern=[[1, P]], base=0, channel_multiplier=0)
        nc.vector.tensor_copy(iota_f[:], ioti[:])

        def build_table(rnd, fsrc, q):
            """tbl quarter = dinv * fsrc; DMA to tb_in (no collective yet)."""
            ts = slice(q * QT, (q + 1) * QT)
            tq = tblp.tile([P, QT, 64], F32, tag="tbl")
            nc.vector.tensor_tensor(
                tq[:], fsrc[:, ts, :],
                dinv_s[:, ts, None].to_broadcast([P, QT, 64]),
                mybir.AluOpType.mult)
            tb_in = tb_ins[rnd]
            nc.sync.dma_start(
                tb_in[q * cfg.qrows:(q + 1) * cfg.qrows]
                .rearrange("(t p) f -> p t f", p=P),
                tq[:])

        def emit_ag(rnd, q):
            """AllGather trigger; sits on the gpsimd queue, so emit it where
            its tb_in quarter is already in flight to avoid stalling gathers."""
            tb_in = tb_ins[rnd]
            nc.gpsimd.collective_compute(
                "AllGather", mybir.AluOpType.bypass,
                replica_groups=[list(range(cfg.nc))],
                ins=[tb_in[q * cfg.qrows:(q + 1) * cfg.qrows]],
                outs=[tb_fulls[rnd][q][:]])

        # ---- phase 1: MLP -> f0 node-major; table quarters fire as covered
        CH = cfg.mm_chunk
        n_mlp = cfg.sp // CH
        q_emitted = 0
        for j in range(n_mlp):
            j0 = j * CH
            xc = xcp.tile([cfg.in_feats, CH], F32, tag="xc")
            # alternate HWDGE queues (SP / ACT) so xT loads stream 2-wide
            (nc.sync if j % 2 == 0 else nc.scalar).dma_start(
                xc[:], xT_d[:, j0:j0 + CH])
            ps1 = psum.tile([H, CH], F32, tag="A")
            nc.tensor.matmul(ps1[:], Ws["W1"][:], xc[:], start=True, stop=True)
            h1c = io.tile([H, CH], F32, tag="h1c")
            nc.scalar.activation(h1c[:], ps1[:], relu, bias=bs["b1"][:])
            ps2 = psum.tile([H, CH], F32, tag="B")
            nc.tensor.matmul(ps2[:], Ws["W2"][:], h1c[:], start=True, stop=True)
            h2c = io.tile([H, CH], F32, tag="h2c")
            nc.scalar.activation(h2c[:], ps2[:], relu, bias=bs["b2"][:])
            for i in range(CH // P):
                t = (j0 + i * P) // P
                ps3 = psum1.tile([P, 64], F32, tag="C")
                nc.tensor.transpose(ps3[:], h2c[:, i * P:(i + 1) * P],
                                    ident[:H, :H])
                nc.scalar.activation(f0[:, t, :], ps3[:], cp)
            while (q_emitted < NQ
                   and (j + 1) * CH >= (q_emitted + 1) * cfg.qrows):
                build_table(0, f0, q_emitted)
                q_emitted += 1
        emit_ag(0, 0)   # round-1 chunk 0; later quarters deferred into bundles

        def emit_filter(j0):
            zl = psum.tile([H, CH], F32, tag="A")
            z1 = psum.tile([H, CH], F32, tag="B")
            z2 = psum1.tile([H, CH], F32, tag="C")
            for i in range(CH // P):
                t = (j0 + i * P) // P
                cs = slice(i * P, (i + 1) * P)
                nc.tensor.matmul(zl[:, cs], f0[:, t, :], sid3[:],
                                 start=True, stop=False)
                nc.tensor.matmul(zl[:, cs], f2[:, t, :], sid075[:],
                                 start=False, stop=True)
                nc.tensor.matmul(z1[:, cs], f1[:, t, :], sid3[:],
                                 start=True, stop=False)
                nc.tensor.matmul(z1[:, cs], f2[:, t, :], sidm15[:],
                                 start=False, stop=True)
                nc.tensor.matmul(z2[:, cs], f2[:, t, :], sid075[:],
                                 start=True, stop=True)
            zlc = io.tile([H, CH], F32, tag="zlc")
            zhc = io.tile([P, CH], F32, tag="zhc")
            nc.scalar.activation(zlc[:], zl[:], cp)
            nc.scalar.activation(zhc[:H, :], z1[:], cp)
            nc.scalar.activation(zhc[H:, :], z2[:], cp)
            pl = psum1.tile([H, CH], F32, tag="C")
            ph = psum.tile([H, CH], F32, tag="A")
            nc.tensor.matmul(pl[:], Ws["W3"][:], zlc[:], start=True, stop=True)
            nc.tensor.matmul(ph[:], Ws["W4"][:], zhc[:], start=True, stop=True)
            ol = io.tile([H, CH], F32, tag="ol")
            oh = io.tile([H, CH], F32, tag="oh")
            nc.scalar.activation(ol[:], pl[:], relu, bias=bs["b3"][:])
            nc.scalar.activation(oh[:], ph[:], relu, bias=bs["b4"][:])
            nc.sync.dma_start(outl_d[:, j0:j0 + CH], ol[:])
            nc.sync.dma_start(outh_d[:, j0:j0 + CH], oh[:])

        # ---- message passing rounds
        last_c = max(c for (c, *_r) in plan)
        chunk_first = {}
        for bi, (c, *_r) in enumerate(plan):
            chunk_first.setdefault(c, bi)
        for rnd, (fprev, fnext) in enumerate([(f0, f1), (f1, f2)]):
            agg = aggp.tile([P, cfg.t, 64], F32, tag="agg")
            nc.gpsimd.memset(agg[:], 0.0)
            q_done = 0
            pending_ag = []   # (due_bundle_idx, rnd, q), emitted in order
            pending_filt = []  # filter col-chunks, drained one per bundle so
                               # their PE bursts don't stall the matmul queue

            def finalize_quarter(q, bi):
                ts = slice(q * QT, (q + 1) * QT)
                nc.vector.tensor_tensor(
                    fnext[:, ts, :], agg[:, ts, :],
                    dinv_s[:, ts, None].to_broadcast([P, QT, 64]),
                    mybir.AluOpType.mult)
                nc.vector.tensor_tensor(fnext[:, ts, :], fprev[:, ts, :],
                                        fnext[:, ts, :],
                                        mybir.AluOpType.subtract)
                if rnd == 0:
                    # filter precompute (3*(f0-f1)) input; and round-2 table
                    nc.vector.tensor_tensor(f0[:, ts, :], f0[:, ts, :],
                                            f1[:, ts, :],
                                            mybir.AluOpType.subtract)
                    build_table(1, f1, q)
                    pending_ag.append((bi + 3, 1, q))
                else:
                    pending_filt.extend(
                        j for j in range(cfg.sp // CH)
                        if (j + 1) * CH <= (q + 1) * cfg.qrows
                        and (q == 0 or (j + 1) * CH > q * cfg.qrows))

            for bi, (c, btiles, goff, events) in enumerate(plan):
                if (rnd == 0 and c < last_c
                        and bi == min(chunk_first[c] + 12,
                                      chunk_first[c + 1] - 1)):
                    # this round's own next-chunk table (built during the MLP)
                    pending_ag.append((bi, 0, c + 1))
                while pending_ag and pending_ag[0][0] <= bi:
                    _, arnd, aq = pending_ag.pop(0)
                    emit_ag(arnd, aq)
                if pending_filt:
                    emit_filter(pending_filt.pop(0) * CH)
                if c == last_c and events:
                    minw = min(ww for ww, _ in events)
                    while q_done < NQ - 1 and minw >= (q_done + 1) * QT:
                        finalize_quarter(q_done, bi)
                        q_done += 1
                gi = idxp.tile([P, cfg.max_span_tiles * 8], I16, tag="gi")
                dv1 = idxp.tile([P, cfg.max_span_tiles], F32, tag="dv1")
                dv2 = idxp.tile([P, cfg.max_span_tiles], F32, tag="dv2")
                nc.sync.dma_start(gi[:, :btiles * 8],
                                  gidx_d[:, goff * 8:(goff + btiles) * 8])
                nc.sync.dma_start(dv1[:, :btiles],
                                  dstw1_d[:, goff:goff + btiles])
                nc.sync.dma_start(dv2[:, :btiles],
                                  dstw2_d[:, goff:goff + btiles])
                gb = gbp.tile([P, cfg.max_span_tiles, 64], F32, tag="gb")
                ni = btiles * P
                nc.gpsimd.dma_gather(
                    gb[:, :btiles, :], tb_fulls[rnd][c][:],
                    gi[:, :btiles * 8], ni, ni, 64)
                ind1 = gbi.tile([P, cfg.max_span_tiles, P], F32, tag="ind1")
                ind2 = gbi.tile([P, cfg.max_span_tiles, P], F32, tag="ind2")
                nc.vector.tensor_tensor(
                    ind1[:, :btiles, :],
                    iota_f[:, None, :].to_broadcast([P, btiles, P]),
                    dv1[:, :btiles, None].to_broadcast([P, btiles, P]),
                    mybir.AluOpType.is_equal)
                need2 = any(s == 2 for _, tl in events for _, s in tl)
                if need2:
                    nc.vector.tensor_tensor(
                        ind2[:, :btiles, :],
                        iota_f[:, None, :].to_broadcast([P, btiles, P]),
                        dv2[:, :btiles, None].to_broadcast([P, btiles, P]),
                        mybir.AluOpType.is_equal)
                for (ww, tl) in events:
                    pw = psum2.tile([P, 64], F32, tag="D")
                    for i, (t, sel) in enumerate(tl):
                        ind = ind1 if sel == 1 else ind2
                        nc.tensor.matmul(pw[:], ind[:, t, :], gb[:, t, :],
                                         start=(i == 0), stop=(i == len(tl) - 1))
                    nc.vector.tensor_tensor(agg[:, ww, :], agg[:, ww, :],
                                            pw[:], mybir.AluOpType.add)
            # finalize the remainder, then flush pending collectives/filters
            while q_done < NQ:
                finalize_quarter(q_done, len(plan))
                q_done += 1
            for (_d, arnd, aq) in pending_ag:
                emit_ag(arnd, aq)
            pending_ag.clear()
            for j in pending_filt:
                emit_filter(j * CH)
            pending_filt.clear()

    nc.compile()
    return nc


# ---------------------------------------------------------------- driver

_CACHE = {}


def run(cfg, inputs, run_fn=None, **spmd_kwargs):
    in_maps, plan, total_tiles, pos_list = preprocess(cfg, **inputs)
    key = (cfg.n_nodes, cfg.n_edges, total_tiles, repr(plan))
    if key not in _CACHE:
        _CACHE[key] = build_nc(cfg, plan, total_tiles)
    nc = _CACHE[key]
    if run_fn is not None:
        results = run_fn(nc, in_maps)
        res = None
    else:
        res = run_bass_kernel_spmd(nc, in_maps, core_ids=list(range(cfg.nc)),
                                   **spmd_kwargs)
        results = res.results
    h_l = np.zeros((cfg.n_nodes, cfg.h), np.float32)
    h_h = np.zeros((cfg.n_nodes, cfg.h), np.float32)
    for c in range(cfg.nc):
        lo = c * cfg.shard
        h_l[lo:lo + cfg.shard] = results[c]["out_l"].T[pos_list[c]]
        h_h[lo:lo + cfg.shard] = results[c]["out_h"].T[pos_list[c]]
    return h_l, h_h, res


def kernel(in_feat, src, dst, W1, b1, W2, b2, W3, b3, W4, b4):
    cfg = Cfg(100000, 1600000, 128, 64, 8)
    h_l, h_h, _ = run(cfg, dict(
        in_feat=np.asarray(in_feat, np.float32),
        src=np.asarray(src, np.int64), dst=np.asarray(dst, np.int64),
        W1=np.asarray(W1, np.float32), b1=np.asarray(b1, np.float32),
        W2=np.asarray(W2, np.float32), b2=np.asarray(b2, np.float32),
        W3=np.asarray(W3, np.float32), b3=np.asarray(b3, np.float32),
        W4=np.asarray(W4, np.float32), b4=np.asarray(b4, np.float32)))
    return h_l, h_h


# revision 46
# speedup vs baseline: 1.2009x; 1.2009x over previous
"""BWGNN (Bernstein-wavelet GNN) Trainium2 kernel, 8-core SPMD.

Sharding: nodes split 8 ways (graph/data parallel); edges partitioned by dst
shard; tiny weights replicated.  Per round of Laplacian message passing the
node-state table (dinv * f) is AllGathered, then per-edge src rows are
fetched with dma_gather (int16 indices, <=1024 per instruction - the HW
SWDGE ring cap).  Segment-sum by dst runs on the TensorEngine via one-hot
indicator matmuls (edges sorted by (src-quarter-chunk, dst-window), packed
contiguously; straddle tiles matmul once per window with self-masking
indicators built by the VectorEngine from iota-vs-dstw is_equal).

Pipeline overlap (this revision): each core's node shard is split into 4
quarters (3125 nodes + 75 zero-pad rows -> 3200 rows/quarter); the table
AllGather runs per (round, quarter) so chunk-c gathers only wait on their
own quarter's collective.  Edge chunks = src quarters.  Round r+1's table
build + AllGather for quarter q fire as soon as round r's last-chunk
bundles pass dst window 25(q+1) (bundles are window-sorted, so the quarter
is final), overlapping the collective and table math with the remaining
gathers.  The output filter/MLP chunks similarly start per quarter while
late round-2 bundles are still in flight.  Pad rows carry dinv=0 so table
pad rows are exactly zero (they double as the gather target for padded
edge slots).

MLP in/out runs feature-major with stationary-weight matmuls; node-major
states are produced by PE transposes; the three Bernstein filters are fused
into scaled-identity matmuls.  Outputs are written feature-major [64, sp];
the host transposes, de-pads and concatenates.

Gather padding is minimized by _balance_positions: per core, nodes are
LPT-packed into their quarter's windows so every window carries ~equal
in-degree per src-chunk; the canonical (chunk, window) group size is a
max over cores, so balancing cuts padded slots 6.3% -> 1.6% (200 bundles
of 1024 indices vs 208).  Within-quarter permutation keeps every node's
chunk membership, so this is a host-side change only.

Measured: 3.709 ms exec (neuron-profile; run-to-run DMA variance ~±0.1ms),
rel err 6.5e-7 vs the jax reference; prior session's baseline was 4.25 ms.
The wall is DMAGatherAnt descriptor generation on the Pool engine, measured
at ~470 ns + ~8.0 ns/index regardless of payload size or locality (2 rounds
x 200k edges/core -> ~3.4 ms busy at ~99% occupancy, zero mid-round gaps;
>1024 idx/instruction crashes the device).  Everything else hides under it:
head ~109 us (MLP quarter 0 + first collective), tail ~78 us.
"""

import sys
from contextlib import ExitStack

import numpy as np

try:
    import concourse  # noqa: F401
except ImportError:  # pragma: no cover
    sys.path.insert(0, "/opt/trn_rl_repo")

import concourse.bacc as bacc
import concourse.bass as bass
import concourse.mybir as mybir
import concourse.tile as tile
from concourse.bass_utils import run_bass_kernel_spmd
from concourse.library_config import mlp
from concourse.masks import make_identity

P = 128
F32 = mybir.dt.float32
I16 = mybir.dt.int16


class Cfg:
    def __init__(self, n_nodes, n_edges, in_feats, h_feats, n_cores,
                 max_span_tiles=8, mm_chunk=512):
        self.n_nodes, self.n_edges = n_nodes, n_edges
        self.in_feats, self.h = in_feats, h_feats
        self.nc = n_cores
        self.shard = n_nodes // n_cores          # 12500
        self.nq = 4                              # quarters per shard
        self.qdata = self.shard // self.nq       # 3125 real rows / quarter
        self.qrows = ((self.qdata + P - 1) // P) * P  # 3200 padded
        self.qt = self.qrows // P                # 25 tiles / quarter
        self.sp = self.qrows * self.nq           # 12800 padded shard
        self.t = self.sp // P                    # 100 node tiles
        self.n_chunks = self.nq                  # gather chunks = src quarters
        self.chunk = self.qrows * n_cores        # 25600 table rows / chunk
        assert self.chunk <= 32000, self.chunk   # int16 index headroom
        self.max_span_tiles = max_span_tiles
        self.mm_chunk = mm_chunk
        assert self.sp % mm_chunk == 0


def _pad_pos(cfg, local):
    """local node id within shard -> padded row position (unbalanced)."""
    return (local // cfg.qdata) * cfg.qrows + (local % cfg.qdata)


def _balance_positions(cfg, dst, src_chunk_of_edge):
    """Per core, permute nodes within their quarter so each of the quarter's
    windows carries ~equal in-degree PER SRC-CHUNK.  The canonical
    (chunk, window) gather-group size is max-over-cores, so balancing all
    four chunk-sums per window cuts the padded gather slots ~4x.  Within-
    quarter permutation keeps every node's own src-chunk membership (and so
    the whole chunk structure) unchanged.  Returns per-core position arrays
    [shard] -> padded position in [0, sp)."""
    shard_of = dst // cfg.shard
    pos_list = []
    wcap = np.full(cfg.qt, P, np.int64)
    wcap[-1] = cfg.qdata - (cfg.qt - 1) * P       # last window: 53 + pads
    for c in range(cfg.nc):
        m = shard_of == c
        dl = (dst[m] - c * cfg.shard).astype(np.int64)
        ch = src_chunk_of_edge[m]
        counts = np.bincount(dl * cfg.nq + ch,
                             minlength=cfg.shard * cfg.nq
                             ).reshape(cfg.shard, cfg.nq).astype(np.int64)
        pos = np.zeros(cfg.shard, np.int64)
        for q in range(cfg.nq):
            lo = q * cfg.qdata
            vecs = counts[lo:lo + cfg.qdata]
            order = np.argsort(-vecs.sum(1), kind="stable")
            sums = np.zeros((cfg.qt, cfg.nq), np.int64)
            fill = np.zeros(cfg.qt, np.int64)
            base = q * cfg.qrows
            for n in order:
                score = (sums + vecs[n]).max(axis=1)
                score[fill >= wcap] = 1 << 40
                w = int(np.argmin(score))
                pos[lo + n] = base + w * P + fill[w]
                fill[w] += 1
                sums[w] += vecs[n]
        pos_list.append(pos)
    return pos_list


# ---------------------------------------------------------------- host prep

def _per_core_groups(cfg, chunk_of, srcrow, dstpos):
    """-> dict (chunk, window) -> (src_row_i16[], dst_in_window_f32[])."""
    out = {}
    for c in range(cfg.n_chunks):
        m = chunk_of == c
        sc = srcrow[m].astype(np.int64)
        dc = dstpos[m].astype(np.int64)
        o = np.argsort(dc, kind="stable")
        sc, dc = sc[o], dc[o]
        w = dc // P
        for ww in np.unique(w):
            sel = w == ww
            out[(c, int(ww))] = (sc[sel].astype(np.int16),
                                 (dc[sel] % P).astype(np.float32))
    return out


def _wrap16(x):
    """flat int16 stream -> [128, n/16]: storage[p, col] = x[col*16 + p%16]."""
    assert len(x) % 16 == 0
    return np.tile(x.reshape(-1, 16).T, (8, 1)).copy()


def preprocess(cfg, in_feat, src, dst, W1, b1, W2, b2, W3, b3, W4, b4):
    n = cfg.n_nodes
    deg = np.bincount(dst, minlength=n).astype(np.float32)
    dinv = np.clip(deg, 1.0, None) ** -0.5

    src_local = src % cfg.shard
    chunk_of_e = src_local // cfg.qdata
    pos_list = _balance_positions(cfg, dst, chunk_of_e)
    pos_g = np.concatenate([pos_list[c] + c * cfg.sp for c in range(cfg.nc)])
    pos_src = pos_g[src] % cfg.sp
    srcrow_e = (src // cfg.shard) * cfg.qrows + (pos_src % cfg.qrows)

    shard_of = dst // cfg.shard
    groups = []
    for c in range(cfg.nc):
        m = shard_of == c
        dstpos = pos_g[dst[m]] - c * cfg.sp
        groups.append(_per_core_groups(cfg, chunk_of_e[m], srcrow_e[m],
                                       dstpos))

    # canonical (chunk, window) sizes = max over cores; groups packed
    # contiguously (tiles may straddle two windows; a straddle tile is
    # matmul'd once per window with self-masking indicators).
    keys = sorted(set().union(*[set(g.keys()) for g in groups]))
    sizes = {k: max(len(g.get(k, ((), ()))[0]) for g in groups) for k in keys}
    layout = {}
    chunk_len = {}
    for c in range(cfg.n_chunks):
        pos = 0
        tile_wins = {}
        for (cc, ww) in keys:
            if cc != c:
                continue
            t0 = pos // P
            if len(tile_wins.get(t0, ())) >= 2 and pos % P:
                pos = (t0 + 1) * P          # avoid 3-window tiles
            layout[(c, ww)] = (pos, pos + sizes[(c, ww)])
            for t in range(pos // P, (pos + sizes[(c, ww)] - 1) // P + 1):
                tile_wins.setdefault(t, []).append(ww)
            pos += sizes[(c, ww)]
        chunk_len[c] = -(-pos // P) * P
        layout[("tw", c)] = tile_wins

    # bundles of <= max_span_tiles tiles per chunk; per-bundle matmul events
    plan = []  # (chunk, btiles, goff_tiles, [(w, [(tile_in_bundle, sel)])])
    goff = 0
    for c in range(cfg.n_chunks):
        tiles = chunk_len[c] // P
        tile_wins = layout[("tw", c)]
        b0 = 0
        while b0 < tiles:
            bt = min(cfg.max_span_tiles, tiles - b0)
            events = {}
            for t in range(b0, b0 + bt):
                for si, ww in enumerate(tile_wins.get(t, [])):
                    events.setdefault(ww, []).append((t - b0, si + 1))
            plan.append((c, bt, goff, sorted(events.items())))
            goff += bt
            b0 += bt
    total_tiles = goff

    in_maps = []
    zero_row = cfg.qdata          # first pad row of core 0, in every chunk

    for core in range(cfg.nc):
        gz = np.zeros(total_tiles * P, np.int16)
        dw1 = np.full(total_tiles * P, 999.0, np.float32)
        dw2 = np.full(total_tiles * P, 999.0, np.float32)
        cbase = {}
        acc_t = 0
        for c in range(cfg.n_chunks):
            cbase[c] = acc_t * P
            acc_t += chunk_len[c] // P
        for (c, ww) in keys:
            start, end = layout[(c, ww)]
            s_arr, d_arr = groups[core].get(
                (c, ww), (np.zeros(0, np.int16), np.zeros(0, np.float32)))
            o = cbase[c] + start
            gz[o:o + (end - start)] = zero_row
            gz[o:o + len(s_arr)] = s_arr
            tile_wins = layout[("tw", c)]
            sl = np.arange(start, end)
            tl = sl // P
            first = np.array([tile_wins[t][0] for t in tl])
            dl_full = np.full(end - start, 999.0 + 128000.0, np.float64)
            dl_full[:len(d_arr)] = d_arr + 128.0 * ww
            v1 = dl_full - 128.0 * first
            v1[(v1 < 0) | (v1 >= P) | (dl_full > 90000)] = 999.0
            second = np.array([tile_wins[t][1] if len(tile_wins[t]) > 1
                               else -999 for t in tl])
            v2 = dl_full - 128.0 * second
            v2[(v2 < 0) | (v2 >= P) | (dl_full > 90000)] = 999.0
            dw1[o:o + (end - start)] = v1
            dw2[o:o + (end - start)] = v2
        lo = core * cfg.shard
        pos = pos_list[core]
        xT = np.zeros((cfg.in_feats, cfg.sp), np.float32)
        xT[:, pos] = in_feat[lo:lo + cfg.shard].T
        full = np.zeros(cfg.sp, np.float32)       # pads carry dinv=0
        full[pos] = dinv[lo:lo + cfg.shard]
        dpm = np.ascontiguousarray(full.reshape(cfg.t, P).T)
        dwt1 = np.ascontiguousarray(dw1.reshape(total_tiles, P).T)
        dwt2 = np.ascontiguousarray(dw2.reshape(total_tiles, P).T)
        in_maps.append({
            "xT": xT, "dinv_pm": dpm,
            "gidx": _wrap16(gz), "dstw1": dwt1, "dstw2": dwt2,
            "W1": np.asarray(W1, np.float32), "W2": np.asarray(W2, np.float32),
            "W3": np.asarray(W3, np.float32), "W4": np.asarray(W4, np.float32),
            "b1": np.asarray(b1, np.float32).reshape(-1, 1),
            "b2": np.asarray(b2, np.float32).reshape(-1, 1),
            "b3": np.asarray(b3, np.float32).reshape(-1, 1),
            "b4": np.asarray(b4, np.float32).reshape(-1, 1),
        })
    return in_maps, plan, total_tiles, pos_list


# ---------------------------------------------------------------- builder

def build_nc(cfg, plan, total_tiles):
    H = cfg.h
    NQ = cfg.nq
    QT = cfg.qt
    idx_cols = total_tiles * 8
    nc = bacc.Bacc("TRN2", target_bir_lowering=False, debug=False,
                   num_devices=cfg.nc)
    xT_d = nc.dram_tensor("xT", [cfg.in_feats, cfg.sp], F32, kind="ExternalInput")
    dinv_d = nc.dram_tensor("dinv_pm", [P, cfg.t], F32, kind="ExternalInput")
    gidx_d = nc.dram_tensor("gidx", [P, idx_cols], I16, kind="ExternalInput")
    dstw1_d = nc.dram_tensor("dstw1", [P, total_tiles], F32, kind="ExternalInput")
    dstw2_d = nc.dram_tensor("dstw2", [P, total_tiles], F32, kind="ExternalInput")
    W_d = {w: nc.dram_tensor(w, [cfg.in_feats if w in ("W1", "W4") else H, H],
                             F32, kind="ExternalInput")
           for w in ("W1", "W2", "W3", "W4")}
    b_d = {b: nc.dram_tensor(b, [H, 1], F32, kind="ExternalInput")
           for b in ("b1", "b2", "b3", "b4")}
    outl_d = nc.dram_tensor("out_l", [H, cfg.sp], F32, kind="ExternalOutput")
    outh_d = nc.dram_tensor("out_h", [H, cfg.sp], F32, kind="ExternalOutput")

    relu = mybir.ActivationFunctionType.Relu
    cp = mybir.ActivationFunctionType.Copy

    with tile.TileContext(nc) as tc, ExitStack() as ctx:
        pers = ctx.enter_context(tc.tile_pool(name="pers", bufs=1))
        dram = ctx.enter_context(tc.tile_pool(name="dram", bufs=1, space="DRAM"))
        io = ctx.enter_context(tc.tile_pool(name="io", bufs=2))
        xcp = ctx.enter_context(tc.tile_pool(name="xcp", bufs=4))
        idxp = ctx.enter_context(tc.tile_pool(name="idxp", bufs=6))
        gbp = ctx.enter_context(tc.tile_pool(name="gbp", bufs=6))
        gbi = ctx.enter_context(tc.tile_pool(name="gbi", bufs=2))
        psum = ctx.enter_context(tc.tile_pool(name="psum", bufs=2, space="PSUM"))
        psum1 = ctx.enter_context(tc.tile_pool(name="psum1", bufs=2, space="PSUM"))
        psum2 = ctx.enter_context(tc.tile_pool(name="psum2", bufs=2, space="PSUM"))

        nc.gpsimd.load_library(mlp)

        tblp = ctx.enter_context(tc.tile_pool(name="tblp", bufs=2))
        aggp = ctx.enter_context(tc.tile_pool(name="aggp", bufs=2))

        f0 = pers.tile([P, cfg.t, 64], F32, tag="f0")
        f1 = pers.tile([P, cfg.t, 64], F32, tag="f1")
        f2 = pers.tile([P, cfg.t, 64], F32, tag="f2")
        dinv_s = pers.tile([P, cfg.t], F32, tag="dinv")
        Ws = {w: pers.tile([cfg.in_feats if w in ("W1", "W4") else H, H],
                           F32, tag=w, name=w + "_s")
              for w in ("W1", "W2", "W3", "W4")}
        bs = {b: pers.tile([H, 1], F32, tag=b, name=b + "_s")
              for b in ("b1", "b2", "b3", "b4")}
        ident = pers.tile([P, P], F32, tag="ident")
        sid3 = pers.tile([P, P], F32, tag="sid3")
        sid075 = pers.tile([P, P], F32, tag="sid075")
        sidm15 = pers.tile([P, P], F32, tag="sidm15")

        tb_ins = [dram.tile([cfg.sp, 64], F32, name=f"tb_in{r}")
                  for r in range(2)]
        tb_fulls = [[dram.tile([cfg.chunk, 64], F32, addr_space="Shared",
                               name=f"tb_full{r}_{q}") for q in range(NQ)]
                    for r in range(2)]
        iota_f = pers.tile([P, P], F32, tag="iota_f")

        for w in Ws:
            nc.sync.dma_start(Ws[w][:], W_d[w][:])
        for b in bs:
            nc.sync.dma_start(bs[b][:], b_d[b][:])
        nc.sync.dma_start(dinv_s[:], dinv_d[:])
        make_identity(nc, ident[:])
        nc.vector.tensor_scalar_mul(sid3[:], ident[:], 3.0)
        nc.vector.tensor_scalar_mul(sid075[:], ident[:], 0.75)
        nc.vector.tensor_scalar_mul(sidm15[:], ident[:], -1.5)
        ioti = pers.tile([P, P], mybir.dt.int32, tag="ioti")
        nc.gpsimd.iota(ioti[:], pattern=[[1, P]], base=0, channel_multiplier=0)
        nc.vector.tensor_copy(iota_f[:], ioti[:])

        def build_table(rnd, fsrc, q):
            """tbl quarter = dinv * fsrc; DMA to tb_in (no collective yet)."""
            ts = slice(q * QT, (q + 1) * QT)
            tq = tblp.tile([P, QT, 64], F32, tag="tbl")
            nc.vector.tensor_tensor(
                tq[:], fsrc[:, ts, :],
                dinv_s[:, ts, None].to_broadcast([P, QT, 64]),
                mybir.AluOpType.mult)
            tb_in = tb_ins[rnd]
            nc.sync.dma_start(
                tb_in[q * cfg.qrows:(q + 1) * cfg.qrows]
                .rearrange("(t p) f -> p t f", p=P),
                tq[:])

        def emit_ag(rnd, q):
            """AllGather trigger; sits on the gpsimd queue, so emit it where
            its tb_in quarter is already in flight to avoid stalling gathers."""
            tb_in = tb_ins[rnd]
            nc.gpsimd.collective_compute(
                "AllGather", mybir.AluOpType.bypass,
                replica_groups=[list(range(cfg.nc))],
                ins=[tb_in[q * cfg.qrows:(q + 1) * cfg.qrows]],
                outs=[tb_fulls[rnd][q][:]])

        # ---- phase 1: MLP -> f0 node-major; table quarters fire as covered
        CH = cfg.mm_chunk
        n_mlp = cfg.sp // CH
        q_emitted = 0
        for j in range(n_mlp):
            j0 = j * CH
            xc = xcp.tile([cfg.in_feats, CH], F32, tag="xc")
            # alternate HWDGE queues (SP / ACT) so xT loads stream 2-wide
            (nc.sync if j % 2 == 0 else nc.scalar).dma_start(
                xc[:], xT_d[:, j0:j0 + CH])
            ps1 = psum.tile([H, CH], F32, tag="A")
            nc.tensor.matmul(ps1[:], Ws["W1"][:], xc[:], start=True, stop=True)
            h1c = io.tile([H, CH], F32, tag="h1c")
            nc.scalar.activation(h1c[:], ps1[:], relu, bias=bs["b1"][:])
            ps2 = psum.tile([H, CH], F32, tag="B")
            nc.tensor.matmul(ps2[:], Ws["W2"][:], h1c[:], start=True, stop=True)
            h2c = io.tile([H, CH], F32, tag="h2c")
            nc.scalar.activation(h2c[:], ps2[:], relu, bias=bs["b2"][:])
            for i in range(CH // P):
                t = (j0 + i * P) // P
                ps3 = psum1.tile([P, 64], F32, tag="C")
                nc.tensor.transpose(ps3[:], h2c[:, i * P:(i + 1) * P],
                                    ident[:H, :H])
                nc.scalar.activation(f0[:, t, :], ps3[:], cp)
            while (q_emitted < NQ
                   and (j + 1) * CH >= (q_emitted + 1) * cfg.qrows):
                build_table(0, f0, q_emitted)
                q_emitted += 1
        emit_ag(0, 0)   # round-1 chunk 0; later quarters deferred into bundles

        def emit_filter(j0):
            zl = psum.tile([H, CH], F32, tag="A")
            z1 = psum.tile([H, CH], F32, tag="B")
            z2 = psum1.tile([H, CH], F32, tag="C")
            for i in range(CH // P):
                t = (j0 + i * P) // P
                cs = slice(i * P, (i + 1) * P)
                nc.tensor.matmul(zl[:, cs], f0[:, t, :], sid3[:],
                                 start=True, stop=False)
                nc.tensor.matmul(zl[:, cs], f2[:, t, :], sid075[:],
                                 start=False, stop=True)
                nc.tensor.matmul(z1[:, cs], f1[:, t, :], sid3[:],
                                 start=True, stop=False)
                nc.tensor.matmul(z1[:, cs], f2[:, t, :], sidm15[:],
                                 start=False, stop=True)
                nc.tensor.matmul(z2[:, cs], f2[:, t, :], sid075[:],
                                 start=True, stop=True)
            zlc = io.tile([H, CH], F32, tag="zlc")
            zhc = io.tile([P, CH], F32, tag="zhc")
            nc.scalar.activation(zlc[:], zl[:], cp)
            nc.scalar.activation(zhc[:H, :], z1[:], cp)
            nc.scalar.activation(zhc[H:, :], z2[:], cp)
            pl = psum1.tile([H, CH], F32, tag="C")
            ph = psum.tile([H, CH], F32, tag="A")
            nc.tensor.matmul(pl[:], Ws["W3"][:], zlc[:], start=True, stop=True)
            nc.tensor.matmul(ph[:], Ws["W4"][:], zhc[:], start=True, stop=True)
            ol = io.tile([H, CH], F32, tag="ol")
            oh = io.tile([H, CH], F32, tag="oh")
            nc.scalar.activation(ol[:], pl[:], relu, bias=bs["b3"][:])
            nc.scalar.activation(oh[:], ph[:], relu, bias=bs["b4"][:])
            nc.sync.dma_start(outl_d[:, j0:j0 + CH], ol[:])
            nc.sync.dma_start(outh_d[:, j0:j0 + CH], oh[:])

        # ---- message passing rounds
        last_c = max(c for (c, *_r) in plan)
        chunk_first = {}
        for bi, (c, *_r) in enumerate(plan):
            chunk_first.setdefault(c, bi)
        for rnd, (fprev, fnext) in enumerate([(f0, f1), (f1, f2)]):
            agg = aggp.tile([P, cfg.t, 64], F32, tag="agg")
            nc.gpsimd.memset(agg[:], 0.0)
            q_done = 0
            pending_ag = []   # (due_bundle_idx, rnd, q), emitted in order
            pending_filt = []  # filter col-chunks, drained one per bundle so
                               # their PE bursts don't stall the matmul queue

            def finalize_quarter(q, bi):
                ts = slice(q * QT, (q + 1) * QT)
                nc.vector.tensor_tensor(
                    fnext[:, ts, :], agg[:, ts, :],
                    dinv_s[:, ts, None].to_broadcast([P, QT, 64]),
                    mybir.AluOpType.mult)
                nc.vector.tensor_tensor(fnext[:, ts, :], fprev[:, ts, :],
                                        fnext[:, ts, :],
                                        mybir.AluOpType.subtract)
                if rnd == 0:
                    # filter precompute (3*(f0-f1)) input; and round-2 table
                    nc.vector.tensor_tensor(f0[:, ts, :], f0[:, ts, :],
                                            f1[:, ts, :],
                                            mybir.AluOpType.subtract)
                    build_table(1, f1, q)
                    pending_ag.append((bi + 3, 1, q))
                else:
                    pending_filt.extend(
                        j for j in range(cfg.sp // CH)
                        if (j + 1) * CH <= (q + 1) * cfg.qrows
                        and (q == 0 or (j + 1) * CH > q * cfg.qrows))

            for bi, (c, btiles, goff, events) in enumerate(plan):
                if (rnd == 0 and c < last_c
                        and bi == min(chunk_first[c] + 12,
                                      chunk_first[c + 1] - 1)):
                    # this round's own next-chunk table (built during the MLP)
                    pending_ag.append((bi, 0, c + 1))
                while pending_ag and pending_ag[0][0] <= bi:
                    _, arnd, aq = pending_ag.pop(0)
                    emit_ag(arnd, aq)
                if pending_filt:
                    emit_filter(pending_filt.pop(0) * CH)
                if c == last_c and events:
                    minw = min(ww for ww, _ in events)
                    while q_done < NQ - 1 and minw >= (q_done + 1) * QT:
                        finalize_quarter(q_done, bi)
                        q_done += 1
                gi = idxp.tile([P, cfg.max_span_tiles * 8], I16, tag="gi")
                dv1 = idxp.tile([P, cfg.max_span_tiles], F32, tag="dv1")
                dv2 = idxp.tile([P, cfg.max_span_tiles], F32, tag="dv2")
                nc.sync.dma_start(gi[:, :btiles * 8],
                                  gidx_d[:, goff * 8:(goff + btiles) * 8])
                nc.sync.dma_start(dv1[:, :btiles],
                                  dstw1_d[:, goff:goff + btiles])
                nc.sync.dma_start(dv2[:, :btiles],
                                  dstw2_d[:, goff:goff + btiles])
                gb = gbp.tile([P, cfg.max_span_tiles, 64], F32, tag="gb")
                ni = btiles * P
                # single_packet=False measured ~128ns faster per 1024-idx
                # gather (microbench S1 vs S4, non-overlapping ranges)
                nc.gpsimd.dma_gather(
                    gb[:, :btiles, :], tb_fulls[rnd][c][:],
                    gi[:, :btiles * 8], ni, ni, 64, single_packet=False)
                ind1 = gbi.tile([P, cfg.max_span_tiles, P], F32, tag="ind1")
                ind2 = gbi.tile([P, cfg.max_span_tiles, P], F32, tag="ind2")
                nc.vector.tensor_tensor(
                    ind1[:, :btiles, :],
                    iota_f[:, None, :].to_broadcast([P, btiles, P]),
                    dv1[:, :btiles, None].to_broadcast([P, btiles, P]),
                    mybir.AluOpType.is_equal)
                need2 = any(s == 2 for _, tl in events for _, s in tl)
                if need2:
                    nc.vector.tensor_tensor(
                        ind2[:, :btiles, :],
                        iota_f[:, None, :].to_broadcast([P, btiles, P]),
                        dv2[:, :btiles, None].to_broadcast([P, btiles, P]),
                        mybir.AluOpType.is_equal)
                for (ww, tl) in events:
                    pw = psum2.tile([P, 64], F32, tag="D")
                    for i, (t, sel) in enumerate(tl):
                        ind = ind1 if sel == 1 else ind2
                        nc.tensor.matmul(pw[:], ind[:, t, :], gb[:, t, :],
                                         start=(i == 0), stop=(i == len(tl) - 1))
                    nc.vector.tensor_tensor(agg[:, ww, :], agg[:, ww, :],
                                            pw[:], mybir.AluOpType.add)
            # finalize the remainder, then flush pending collectives/filters
            while q_done < NQ:
                finalize_quarter(q_done, len(plan))
                q_done += 1
            for (_d, arnd, aq) in pending_ag:
                emit_ag(arnd, aq)
            pending_ag.clear()
            for j in pending_filt:
                emit_filter(j * CH)
            pending_filt.clear()

    nc.compile()
    return nc


# ---------------------------------------------------------------- driver

_CACHE = {}


def run(cfg, inputs, run_fn=None, **spmd_kwargs):
    in_maps, plan, total_tiles, pos_list = preprocess(cfg, **inputs)
    key = (cfg.n_nodes, cfg.n_edges, total_tiles, repr(plan))
    if key not in _CACHE:
        _CACHE[key] = build_nc(cfg, plan, total_tiles)
    nc = _CACHE[key]
    if run_fn is not None:
        results = run_fn(nc, in_maps)
        res = None
    else:
        res = run_bass_kernel_spmd(nc, in_maps, core_ids=list(range(cfg.nc)),
                                   **spmd_kwargs)
        results = res.results
    h_l = np.zeros((cfg.n_nodes, cfg.h), np.float32)
    h_h = np.zeros((cfg.n_nodes, cfg.h), np.float32)
    for c in range(cfg.nc):
        lo = c * cfg.shard
        h_l[lo:lo + cfg.shard] = results[c]["out_l"].T[pos_list[c]]
        h_h[lo:lo + cfg.shard] = results[c]["out_h"].T[pos_list[c]]
    return h_l, h_h, res


def kernel(in_feat, src, dst, W1, b1, W2, b2, W3, b3, W4, b4):
    cfg = Cfg(100000, 1600000, 128, 64, 8)
    h_l, h_h, _ = run(cfg, dict(
        in_feat=np.asarray(in_feat, np.float32),
        src=np.asarray(src, np.int64), dst=np.asarray(dst, np.int64),
        W1=np.asarray(W1, np.float32), b1=np.asarray(b1, np.float32),
        W2=np.asarray(W2, np.float32), b2=np.asarray(b2, np.float32),
        W3=np.asarray(W3, np.float32), b3=np.asarray(b3, np.float32),
        W4=np.asarray(W4, np.float32), b4=np.asarray(b4, np.float32)))
    return h_l, h_h
